# revision 64
# baseline (speedup 1.0000x reference)
"""FluxSingleTransformerBlock on 8 trn2 NeuronCores — v2.

Sharding: tensor-parallel over heads (3/core) and mlp_hidden (1536/core);
out-proj row-parallel with gate folded into the local partial, chunked bf16
ReduceScatter along the OUTPUT-FEATURE dim overlapped with compute; final
residual add on each core's strided feature shard; host re-assembles.

v2 changes vs baseline:
- nx kept resident in SBUF (no nx_dram round trip); qkv/mlp read from SBUF.
- qkv projections in fp8e4m3 (x64 weight prescale) with DoubleRow perf mode
  (2 k-tiles per pass).
- emb computed with temb-stationary matmuls (72xN=384 instead of 216xN=1);
  AllGather overlapped under the LN-stats phase.
- LN-stats squares split between Scalar and Vector engines.
- attention exp at [128,1024] (2 kt per activation); v transposed inline.
- out-proj D-sharded ReduceScatter with gate+bias folded into the PSUM
  copy; final phase is a single add per chunk, overlapped with RS chunks.
- weight DMAs issued early into long-lived pools.
"""

import os
import sys

for _p in ("/opt/trn_rl_repo", "/root/.axon_site/_ro/trn_rl_repo"):
    if os.path.isdir(_p) and _p not in sys.path:
        sys.path.append(_p)

import numpy as np

import concourse.bass as bass
import concourse.bacc as bacc
import concourse.mybir as mybir
import concourse.tile as tile
import concourse.masks as masks
from concourse.bass_utils import run_bass_kernel_spmd

FP32 = mybir.dt.float32
BF16 = mybir.dt.bfloat16
FP8 = mybir.dt.float8e4
AF = mybir.ActivationFunctionType
ALU = mybir.AluOpType
DR = mybir.MatmulPerfMode.DoubleRow

B, S, D, H, DH = 1, 2048, 3072, 24, 128
M = 4 * D
NC = 8
HPC = H // NC            # heads per core = 3
DQ = HPC * DH            # q/k/v cols per core = 384
MPC = M // NC            # mlp rows per core = 1536
MT = MPC // 128          # 12
NPC = 3 * D // NC        # norm rows per core = 1152
FPC = DQ + MPC           # fused contraction rows per core = 1920
FT = FPC // 128          # 15
KT = D // 128            # 24 d_in tiles
KP = KT // 2             # 12 k-tile pairs (DoubleRow)
ST = S // 128            # 16 seq tiles
CW = 512                 # seq chunk width
NCH = S // CW            # 4
DBLK = D // 128          # 24 out-proj feature blocks
RSC = 12                 # reduce-scatter chunks (2 dblks each)
RB = DBLK // RSC         # 2 dblk per rs chunk
RWC = RB * 128 // NC     # 32 rows per core per rs chunk
EPS = 1e-6
ISQD = float(1.0 / np.sqrt(DH))
WS = 64.0                # fp8 weight prescale
FP8_QKV = True

_CACHE = {}


def _build(debug=False):
    key = ("nc", debug)
    if key in _CACHE:
        return _CACHE[key]
    nc = bacc.Bacc("TRN2", target_bir_lowering=False, debug=False, num_devices=NC)

    def din(name, shape, dt=BF16):
        return nc.dram_tensor(name, list(shape), dt, kind="ExternalInput").ap()

    wdt = FP8 if FP8_QKV else BF16
    io = {
        "xTb": din("xTb", [D, S]),
        "xresT": din("xresT", [RSC, RWC, S], FP32),
        "tembT": din("tembT", [128, KT], FP32),
        "outb_col": din("outb_col", [128, DBLK], FP32),
        "normT": din("normT", [D, NPC]),
        "normb_row": din("normb_row", [1, NPC]),
        "qw8": din("qw8", [KP, 128, 2, DQ], wdt),
        "kw8": din("kw8", [KP, 128, 2, DQ], wdt),
        "vw8": din("vw8", [KP, 128, 2, DQ], wdt),
        "qb": din("qb", [128, HPC], FP32),
        "kb": din("kb", [128, HPC], FP32),
        "vb": din("vb", [128, HPC], FP32),
        "mlpT": din("mlpT", [MT, 128, D]),
        "mlpb": din("mlpb", [128, MT], FP32),
        "outTd": din("outTd", [DBLK, 128, FT, 128]),
        "cosq": din("cosq", [128, S]),
        "sinq": din("sinq", [128, S]),
        "cosk": din("cosk", [128, S]),
        "sink": din("sink", [128, S]),
        "y": nc.dram_tensor("y", [RSC, RWC, S], FP32, kind="ExternalOutput").ap(),
    }
    io["dbg"] = {}
    if debug:
        def dodbg(name, shape, dt=BF16):
            return nc.dram_tensor(name, list(shape), dt, kind="ExternalOutput").ap()
        io["dbg"] = {
            "d_emb": dodbg("d_emb", [1, NPC], FP32),
            "d_mu": dodbg("d_mu", [128, S], FP32),
            "d_rstd": dodbg("d_rstd", [128, S], FP32),
            "d_nx0": dodbg("d_nx0", [128, S]),
            "d_q0": dodbg("d_q0", [128, S]),
            "d_k0": dodbg("d_k0", [128, S]),
            "d_v0": dodbg("d_v0", [128, S]),
            "d_o0": dodbg("d_o0", [128, S]),
            "d_g0": dodbg("d_g0", [128, S]),
        }

    with tile.TileContext(nc) as tc:
        _emit(tc, io)
    nc.compile()
    _CACHE[key] = nc
    return nc


def _emit(tc, io):
    from contextlib import ExitStack

    with ExitStack() as ctx:
        _emit_body(ctx, tc, io)


def _emit_body(ctx, tc, io):
    nc = tc.nc
    dbg = io["dbg"]
    debug = bool(dbg)

    def dma(out, in_):
        nc.sync.dma_start(out=out, in_=in_)

    # ---------- constants + early weight prefetch ----------
    const = ctx.enter_context(tc.tile_pool(name="const", bufs=1))
    ones_b = const.tile([128, 128], BF16)
    nc.vector.memset(ones_b[:], 1.0)
    ident_b = const.tile([128, 128], BF16)
    masks.make_identity(nc, ident_b[:])
    tembT_t = const.tile([128, KT], FP32)
    dma(tembT_t[:], io["tembT"][:, :])
    qb_t = const.tile([128, HPC], FP32)
    kb_t = const.tile([128, HPC], FP32)
    vb_t = const.tile([128, HPC], FP32)
    dma(qb_t[:], io["qb"][:, :])
    dma(kb_t[:], io["kb"][:, :])
    dma(vb_t[:], io["vb"][:, :])
    mlpb_t = const.tile([128, MT], FP32)
    dma(mlpb_t[:], io["mlpb"][:, :])
    normb_t = const.tile([1, NPC], BF16)
    dma(normb_t[:], io["normb_row"][:, :])
    outbc_t = const.tile([128, DBLK], FP32)
    dma(outbc_t[:], io["outb_col"][:, :])
    eps_t = const.tile([128, 1], FP32)
    nc.vector.memset(eps_t[:], EPS)

    dram = ctx.enter_context(tc.tile_pool(name="dram", bufs=1, space="DRAM"))
    nx_dram = dram.tile([KT, 128, S], BF16, tag="nxd", name="nx_dram")
    adaln = ctx.enter_context(tc.tile_pool(name="adaln", bufs=1))
    shift_c = adaln.tile([128, KT], FP32)
    scale1_c = adaln.tile([128, KT], FP32)
    gate_c = adaln.tile([128, KT], FP32)
    gob_c = adaln.tile([128, DBLK], FP32)

    # right-side stack: long-lived tensors with interleaved lifetimes.
    # creation order: qkv, vt (released end-P4); vbuf (end of v-transposes);
    # cossin, wqkv, stats (end-P2); wnorm (post-emb).  Releases are LIFO.
    qkvp_cm = tc.tile_pool(name="qkv", bufs=1, side="right")
    qkvp = qkvp_cm.__enter__()
    qT = [qkvp.tile([128, S], BF16, tag=f"q{m}", name=f"q{m}") for m in range(HPC)]
    kT = [qkvp.tile([128, S], BF16, tag=f"k{m}", name=f"k{m}") for m in range(HPC)]
    vtp_cm = tc.tile_pool(name="vt", bufs=1, side="right")
    vtp = vtp_cm.__enter__()
    vts = {}
    for h in range(HPC):
        for kt in range(ST):
            vts[(h, kt)] = vtp.tile([128, 128], BF16, tag=f"vts{h}_{kt}",
                                    name=f"vts{h}_{kt}")
    pvb_cm = tc.tile_pool(name="vbuf", bufs=1, side="right")
    pvb = pvb_cm.__enter__()
    vT = [pvb.tile([128, S], BF16, tag=f"v{m}", name=f"v{m}") for m in range(HPC)]

    pcs_cm = tc.tile_pool(name="cossin", bufs=1, side="right")
    pcs = pcs_cm.__enter__()
    cosq_t = pcs.tile([128, S], BF16, tag="cosq")
    sinq_t = pcs.tile([128, S], BF16, tag="sinq")
    cosk_t = pcs.tile([128, S], BF16, tag="cosk")
    sink_t = pcs.tile([128, S], BF16, tag="sink")

    wdt = FP8 if FP8_QKV else BF16
    pw_qkv_cm = tc.tile_pool(name="wqkv", bufs=3 * KP, side="right")
    pw_qkv = pw_qkv_cm.__enter__()
    wq, wk, wv = [], [], []
    for name, lst in (("qw8", wq), ("kw8", wk), ("vw8", wv)):
        for tp in range(KP):
            w = pw_qkv.tile([128, 2, DQ], wdt, tag="wqkv8")
            lst.append(w)

    stats_cm = tc.tile_pool(name="stats", bufs=1, side="right")
    stats = stats_cm.__enter__()
    RSTDb = [stats.tile([128, S // 2], BF16, tag=f"rstdb{h}",
                        name=f"rstdb{h}") for h in range(2)]
    MUb = [stats.tile([128, S // 2], BF16, tag=f"mub{h}", name=f"mub{h}")
           for h in range(2)]

    # norm weights: resident until emb done.  These DMAs go first — the
    # emb -> AllGather chain is the startup critical path.
    pnw_cm = tc.tile_pool(name="wnorm", bufs=KT, side="right")
    pnw = pnw_cm.__enter__()
    wn = []
    for t in range(KT):
        w = pnw.tile([128, NPC], BF16, tag="nslab")
        dma(w[:], io["normT"][bass.ts(t, 128), :])
        wn.append(w)
    dma(cosq_t[:], io["cosq"][:, :])
    dma(sinq_t[:], io["sinq"][:, :])
    dma(cosk_t[:], io["cosk"][:, :])
    dma(sink_t[:], io["sink"][:, :])

    # ---------- phase 0: emb = silu(temb) @ norm_wT (temb stationary) ----------
    with (
        tc.tile_pool(name="p0", bufs=1) as p0,
        tc.tile_pool(name="p0p", bufs=3, space="PSUM") as p0p,
    ):
        st_t = p0.tile([128, KT], BF16)
        nc.scalar.activation(st_t[:], tembT_t[:], AF.Silu)
        emb_sb = p0.tile([1, NPC], FP32)
        for j3 in range(NPC // 384):
            eps_ps = p0p.tile([1, 384], FP32, tag="embps")
            for t in range(KT):
                nc.tensor.matmul(
                    eps_ps[0:1, :],
                    st_t[:, t : t + 1],
                    wn[t][:, bass.ds(j3 * 384, 384)],
                    start=(t == 0), stop=(t == KT - 1),
                )
            nc.vector.tensor_add(
                emb_sb[0:1, bass.ds(j3 * 384, 384)], eps_ps[0:1, :],
                normb_t[0:1, bass.ds(j3 * 384, 384)],
            )
        if debug:
            dma(dbg["d_emb"][:, :], emb_sb[:])
        emb_loc = dram.tile([1, NPC], FP32, tag="embloc")
        dma(emb_loc[:, :], emb_sb[0:1, :])
        emb_all = dram.tile([3 * KT, 128], FP32, tag="emball")
        nc.gpsimd.collective_compute(
            "AllGather",
            ALU.bypass,
            replica_groups=[list(range(NC))],
            ins=[emb_loc.opt()],
            outs=[emb_all.opt()],
        )
        dma(shift_c[:], emb_all[0:KT, :].rearrange("t p -> p t"))
        dma(scale1_c[:], emb_all[KT : 2 * KT, :].rearrange("t p -> p t"))
        dma(gate_c[:], emb_all[2 * KT : 3 * KT, :].rearrange("t p -> p t"))
        nc.vector.tensor_scalar_add(scale1_c[:], scale1_c[:], 1.0)
        # gob = gate * out_b / NC  (per-partition bias folded pre-ReduceScatter)
        nc.vector.scalar_tensor_tensor(
            gob_c[:], gate_c[:], 1.0 / NC, outbc_t[:],
            op0=ALU.mult, op1=ALU.mult,
        )

    # ---------- phase 1: layernorm stats (overlaps AllGather) ----------
    with (
        tc.tile_pool(name="p1", bufs=1) as p1,
        tc.tile_pool(name="p1x", bufs=12) as p1x,
        tc.tile_pool(name="p1s", bufs=3) as p1s,
        tc.tile_pool(name="p1p", bufs=2, space="PSUM") as p1p,
    ):
        HW = S // 2
        for half in range(2):
            hsl = bass.ds(half * HW, HW)
            sum_ps = p1p.tile([128, HW], FP32, tag="sum")
            ssq_ps = p1p.tile([128, HW], FP32, tag="ssq")
            for t in range(KT):
                xt = p1x.tile([128, HW], BF16, tag="xin")
                dma(xt[:], io["xTb"][bass.ts(t, 128), hsl])
                sq = p1s.tile([128, HW], BF16, tag="xsq")
                if t % 2 == 0:
                    nc.scalar.activation(sq[:], xt[:], AF.Square)
                else:
                    nc.vector.tensor_mul(sq[:], xt[:], xt[:])
                for cc in range(2):
                    csl = bass.ts(cc, CW)
                    nc.tensor.matmul(
                        sum_ps[:, csl], ones_b[:], xt[:, csl],
                        start=(t == 0), stop=(t == KT - 1),
                    )
                    nc.tensor.matmul(
                        ssq_ps[:, csl], ones_b[:], sq[:, csl],
                        start=(t == 0), stop=(t == KT - 1),
                    )
            if half == 0:
                # prefetch qkv weights while half-1 stats run
                for name, lst in (("qw8", wq), ("kw8", wk), ("vw8", wv)):
                    for tp in range(KP):
                        dma(lst[tp][:], io[name][tp, :, :, :])
            mu = p1.tile([128, HW], FP32, tag="mu")
            nc.scalar.activation(mu[:], sum_ps[:], AF.Copy, scale=1.0 / D)
            musq = p1.tile([128, HW], FP32, tag="musq")
            nc.vector.scalar_tensor_tensor(
                musq[:], mu[:], 0.0, mu[:], op0=ALU.bypass, op1=ALU.mult
            )
            nc.vector.tensor_scalar_add(musq[:], musq[:], -EPS)
            vare = p1.tile([128, HW], FP32, tag="vare")
            nc.vector.scalar_tensor_tensor(
                vare[:], ssq_ps[:], 1.0 / D, musq[:],
                op0=ALU.mult, op1=ALU.subtract,
            )
            rinv = p1.tile([128, HW], FP32, tag="rinv")
            rstd = p1.tile([128, HW], FP32, tag="rstd")
            nc.vector.reciprocal_approx_accurate(rinv[:], vare[:], rstd[:])
            nc.scalar.activation(rstd[:], rinv[:], AF.Sqrt)
            nc.vector.tensor_copy(RSTDb[half][:], rstd[:])
            nc.vector.tensor_copy(MUb[half][:], mu[:])
            if debug:
                dma(dbg["d_mu"][:, hsl], mu[:])
                dma(dbg["d_rstd"][:, hsl], rstd[:])

    pnw_cm.__exit__(None, None, None)  # free norm weights

    # ---------- phase 2: LN apply + qkv (fp8 DoubleRow) + rms + rope ----------
    # engine split: DVE (sub, fp8-copy, rope-half), GpSimd (mul, rope-half),
    # ACT (scale/shift apply, rms pieces).
    W2 = 2 * CW
    with (
        tc.tile_pool(name="p2x", bufs=3) as p2x,
        tc.tile_pool(name="p2n8", bufs=2) as p2n8,
        tc.tile_pool(name="p2", bufs=2) as p2,
        tc.tile_pool(name="p2r", bufs=1) as p2r,
        tc.tile_pool(name="p2f", bufs=1) as p2f,
        tc.tile_pool(name="p2p", bufs=3, space="PSUM") as p2p,
        tc.tile_pool(name="p2p2", bufs=1, space="PSUM") as p2p2,
    ):
        for cp in range(S // W2):
            csl = bass.ds(cp * W2, W2)
            nx8 = p2n8.tile([128, KT, W2], FP8 if FP8_QKV else BF16, tag="nx8")
            for t in range(KT):
                xt = p2x.tile([128, W2], BF16, tag="xin2")
                dma(xt[:], io["xTb"][bass.ts(t, 128), csl])
                u = p2x.tile([128, W2], BF16, tag="u")
                nc.vector.tensor_sub(u[:], xt[:], MUb[cp][:])
                u2 = p2x.tile([128, W2], BF16, tag="u2")
                nc.gpsimd.tensor_mul(u2[:], u[:], RSTDb[cp][:])
                nxb = p2x.tile([128, W2], BF16, tag="nxb")
                if t % 4 == 0:
                    nc.vector.tensor_scalar(
                        nxb[:], u2[:], scale1_c[:, t : t + 1],
                        shift_c[:, t : t + 1], op0=ALU.mult, op1=ALU.add,
                    )
                else:
                    nc.scalar.activation(
                        nxb[:], u2[:], AF.Identity,
                        bias=shift_c[:, t : t + 1],
                        scale=scale1_c[:, t : t + 1],
                    )
                nc.scalar.activation(nx8[:, t, :], nxb[:], AF.Identity)
                dma(nx_dram[t, :, csl], nxb[:])
                if debug and t == 0:
                    dma(dbg["d_nx0"][:, csl], nxb[:])
            for pi, (wts, bt) in enumerate(((wq, qb_t), (wk, kb_t), (wv, vb_t))):
                for m in range(HPC):
                    ps = p2p.tile([128, W2], FP32, tag="qkvps")
                    for c2 in range(2):
                        c2s = bass.ts(c2, CW)
                        if FP8_QKV:
                            for tp in range(KP):
                                nc.tensor.matmul(
                                    ps[:, c2s],
                                    wts[tp][:, :, bass.ts(m, 128)],
                                    nx8[:, 2 * tp : 2 * tp + 2, c2s],
                                    start=(tp == 0), stop=(tp == KP - 1),
                                    perf_mode=DR,
                                )
                        else:
                            for tp in range(KP):
                                for i in range(2):
                                    nc.tensor.matmul(
                                        ps[:, c2s],
                                        wts[tp][:, i, bass.ts(m, 128)],
                                        nx8[:, 2 * tp + i, c2s],
                                        start=(tp == 0 and i == 0),
                                        stop=(tp == KP - 1 and i == 1),
                                    )
                    inv = 1.0 / WS if FP8_QKV else 1.0
                    if pi == 2:
                        nc.scalar.activation(
                            vT[m][:, csl], ps[:], AF.Identity,
                            bias=bt[:, m : m + 1], scale=inv,
                        )
                        if debug and m == 0:
                            dma(dbg["d_v0"][:, csl], vT[m][:, csl])
                        continue
                    qsb = p2.tile([128, W2], BF16, tag="qsb")
                    nc.scalar.activation(
                        qsb[:], ps[:], AF.Identity,
                        bias=bt[:, m : m + 1], scale=inv,
                    )
                    sq = p2.tile([128, W2], BF16, tag="sq2")
                    nc.scalar.activation(
                        sq[:], ps[:], AF.Square,
                        bias=bt[:, m : m + 1], scale=inv,
                    )
                    ssq = p2p2.tile([128, W2], FP32, tag="ssq2")
                    for c2 in range(2):
                        c2s = bass.ts(c2, CW)
                        nc.tensor.matmul(
                            ssq[:, c2s], ones_b[:], sq[:, c2s],
                            start=True, stop=True,
                        )
                    vare = p2f.tile([128, W2], FP32, tag="vare2")
                    nc.scalar.activation(
                        vare[:], ssq[:], AF.Identity, bias=eps_t[:, 0:1],
                        scale=1.0 / DH,
                    )
                    rinv = p2f.tile([128, W2], FP32, tag="rinv2")
                    rst = p2f.tile([128, W2], FP32, tag="rst2")
                    nc.vector.reciprocal_approx_accurate(rinv[:], vare[:], rst[:])
                    nc.scalar.activation(rst[:], rinv[:], AF.Sqrt)
                    # rope first (linear in q), rms_w folded into cos/sin on
                    # host, rms 1/rms scale applied last
                    qsw = p2r.tile([128, W2], BF16, tag="qsw")
                    nc.vector.tensor_copy(qsw[0:64, :], qsb[64:128, :])
                    nc.vector.tensor_copy(qsw[64:128, :], qsb[0:64, :])
                    te = p2r.tile([128, W2], BF16, tag="te")
                    to = p2r.tile([128, W2], BF16, tag="to")
                    ct, st_ = (cosq_t, sinq_t) if pi == 0 else (cosk_t, sink_t)
                    nc.vector.tensor_mul(te[:], qsb[:], ct[:, csl])
                    nc.vector.tensor_mul(to[:], qsw[:], st_[:, csl])
                    tsum = p2r.tile([128, W2], BF16, tag="tsum")
                    nc.gpsimd.tensor_add(tsum[:], te[:], to[:])
                    dst = qT[m] if pi == 0 else kT[m]
                    nc.vector.tensor_mul(dst[:, csl], tsum[:], rst[:])
        if debug:
            dma(dbg["d_q0"][:, :], qT[0][:])
            dma(dbg["d_k0"][:, :], kT[0][:])

    stats_cm.__exit__(None, None, None)
    pw_qkv_cm.__exit__(None, None, None)
    pcs_cm.__exit__(None, None, None)

    # v transposes (overlap the mlp phase)
    with tc.tile_pool(name="pvt", bufs=4, space="PSUM") as pvt:
        for h in range(HPC):
            for kt in range(ST):
                tp_ps = pvt.tile([128, 128], BF16, tag="vtp")
                nc.tensor.transpose(
                    tp_ps[:], vT[h][:, bass.ts(kt, 128)], ident_b[:]
                )
                nc.scalar.activation(vts[(h, kt)][:], tp_ps[:], AF.Copy)
    pvb_cm.__exit__(None, None, None)

    # ---------- phase 3: mlp -> gT (nx streamed from DRAM, chunk pairs) ----
    gp_cm = tc.tile_pool(name="g", bufs=1)
    gp = gp_cm.__enter__()
    gT = [gp.tile([128, S], BF16, tag=f"g{m}", name=f"g{m}") for m in range(MT)]
    with (
        tc.tile_pool(name="p3w", bufs=3) as p3w,
        tc.tile_pool(name="p3n", bufs=KT + 4) as p3n,
        tc.tile_pool(name="p3p", bufs=2, space="PSUM") as p3p,
    ):
        W2 = 2 * CW
        for cp in range(NCH // 2):
            cpsl = bass.ds(cp * W2, W2)
            nxc = []
            for t in range(KT):
                nt = p3n.tile([128, W2], BF16, tag="nxc3")
                dma(nt[:], nx_dram[t, :, cpsl])
                nxc.append(nt)
            for m in range(MT):
                wm = p3w.tile([128, D], BF16, tag="wmlp")
                dma(wm[:], io["mlpT"][m, :, :])
                ps = p3p.tile([128, W2], FP32, tag="mlpps")
                for t in range(KT):
                    for c2 in range(2):
                        nc.tensor.matmul(
                            ps[:, bass.ts(c2, CW)], wm[:, bass.ts(t, 128)],
                            nxc[t][:, bass.ts(c2, CW)],
                            start=(t == 0), stop=(t == KT - 1),
                        )
                nc.scalar.activation(
                    gT[m][:, cpsl], ps[:], AF.Gelu_apprx_tanh,
                    bias=mlpb_t[:, m : m + 1],
                )
        if debug:
            dma(dbg["d_g0"][:, :], gT[0][:])

    # ---------- phase 4: attention -> oT ----------
    op_cm = tc.tile_pool(name="o", bufs=1)
    op = op_cm.__enter__()
    oT = [op.tile([128, S], BF16, tag=f"o{m}", name=f"o{m}") for m in range(HPC)]
    with (
        tc.tile_pool(name="p4", bufs=2) as p4,
        tc.tile_pool(name="p4pt", bufs=10) as p4pt,
        tc.tile_pool(name="p4p", bufs=2, space="PSUM") as p4p,
        tc.tile_pool(name="p4pa", bufs=2, space="PSUM") as p4pa,
    ):
        for h in range(HPC):
            for c in range(NCH):
                csl = bass.ts(c, CW)
                pts = []
                for g in range(ST // 2):
                    sps = p4p.tile([128, 1024], FP32, tag="sps")
                    for i in range(2):
                        kt = 2 * g + i
                        nc.tensor.matmul(
                            sps[:, bass.ts(i, CW)],
                            kT[h][:, bass.ts(kt, 128)], qT[h][:, csl],
                            start=True, stop=True,
                        )
                    pt = p4pt.tile([128, 1024], BF16, tag="pt")
                    nc.scalar.activation(pt[:], sps[:], AF.Exp, scale=ISQD)
                    pts.append(pt)
                ops = p4pa.tile([128, CW], FP32, tag="ops")
                sms = p4pa.tile([128, CW], FP32, tag="sms")
                for kt in range(ST):
                    nc.tensor.matmul(
                        ops[:], vts[(h, kt)][:],
                        pts[kt // 2][:, bass.ts(kt % 2, CW)],
                        start=(kt == 0), stop=(kt == ST - 1),
                    )
                for kt in range(ST):
                    nc.tensor.matmul(
                        sms[:], ones_b[:],
                        pts[kt // 2][:, bass.ts(kt % 2, CW)],
                        start=(kt == 0), stop=(kt == ST - 1),
                    )
                rec = p4.tile([128, CW], FP32, tag="rec")
                scr = p4.tile([128, CW], FP32, tag="scr4")
                nc.vector.reciprocal_approx_accurate(rec[:], sms[:], scr[:])
                nc.vector.tensor_mul(oT[h][:, csl], ops[:], rec[:])
        if debug:
            dma(dbg["d_o0"][:, :], oT[0][:])

    vtp_cm.__exit__(None, None, None)
    qkvp_cm.__exit__(None, None, None)

    # ---------- phase 5: out proj (D-sharded) + chunked ReduceScatter ------
    fusedT = oT + gT
    with (
        tc.tile_pool(name="p5w", bufs=3) as p5w,
        tc.tile_pool(name="p5", bufs=4) as p5,
        tc.tile_pool(name="p5p", bufs=3, space="PSUM") as p5p,
        tc.tile_pool(name="p6", bufs=2) as p6,
    ):
        for i in range(RSC):
            rs_in_i = dram.tile([RB * 128, S], BF16, tag=f"rsin{i}",
                                name=f"rsin{i}")
            rs_out_i = dram.tile([RWC, S], BF16, tag=f"rsout{i}",
                                 name=f"rsout{i}")
            for db in range(RB):
                dblk = i * RB + db
                wo = p5w.tile([128, FT, 128], BF16, tag="wout")
                dma(wo[:], io["outTd"][dblk, :, :, :])
                for scp in range(NCH // 2):
                    ps = p5p.tile([128, 2 * CW], FP32, tag="ops5")
                    for c2 in range(2):
                        ssl = bass.ds(scp * 2 * CW + c2 * CW, CW)
                        for f in range(FT):
                            nc.tensor.matmul(
                                ps[:, bass.ts(c2, CW)], wo[:, f, :],
                                fusedT[f][:, ssl],
                                start=(f == 0), stop=(f == FT - 1),
                            )
                    osb = p5.tile([128, 2 * CW], BF16, tag="osb")
                    nc.scalar.activation(
                        osb[:], ps[:], AF.Identity,
                        bias=gob_c[:, dblk : dblk + 1],
                        scale=gate_c[:, dblk : dblk + 1],
                    )
                    dma(rs_in_i[bass.ts(db, 128), bass.ts(scp, 2 * CW)],
                        osb[:])
            nc.gpsimd.collective_compute(
                "ReduceScatter",
                ALU.add,
                replica_groups=[list(range(NC))],
                ins=[rs_in_i.opt()],
                outs=[rs_out_i.opt()],
            )
            # ---------- phase 6 (progressive): residual add on own shard ----
            sh = p6.tile([RWC, S], BF16, tag="shard")
            dma(sh[:], rs_out_i[:, :])
            xr = p6.tile([RWC, S], FP32, tag="xr")
            dma(xr[:], io["xresT"][i, :, :])
            yv = p6.tile([RWC, S], FP32, tag="yv")
            nc.vector.tensor_add(yv[:], sh[:], xr[:])
            dma(io["y"][i, :, :], yv[:])

    op_cm.__exit__(None, None, None)
    gp_cm.__exit__(None, None, None)


# ======================= host side =======================================

def _bf16(a):
    import ml_dtypes
    return np.ascontiguousarray(np.asarray(a).astype(ml_dtypes.bfloat16))


def _fp8(a, scale):
    import ml_dtypes
    y = np.clip(np.asarray(a, np.float32) * scale, -240.0, 240.0)
    return np.ascontiguousarray(y.astype(ml_dtypes.float8_e4m3))


def _pack_pairs(wT, cols, scale):
    """wT [D, ncols-sel] -> [KP, 128, 2, ncols] DoubleRow pair layout."""
    w = np.asarray(wT, np.float32)[:, cols]          # [D, n]
    n = w.shape[1]
    w = w.reshape(KP, 2, 128, n).transpose(0, 2, 1, 3)  # [KP, 128, 2, n]
    if FP8_QKV:
        return _fp8(w, scale)
    return _bf16(w)


def _prep_inputs(hidden_states, temb, rope_cos, rope_sin, norm_w, norm_b,
                 qw, qb, kw, kb, vw, vb, q_rms_w, k_rms_w, mlp_w, mlp_b,
                 out_w, out_b):
    f32 = np.float32
    x = np.ascontiguousarray(np.asarray(hidden_states).reshape(S, D).astype(f32))
    xT = x.T  # [D, S]
    xTb = _bf16(xT)
    perm = np.concatenate([np.arange(0, DH, 2), np.arange(1, DH, 2)])
    cosH = np.asarray(rope_cos).astype(f32)[:, 0::2].T
    sinH = np.asarray(rope_sin).astype(f32)[:, 0::2].T
    cosP = np.concatenate([cosH, cosH], 0)
    sinP = np.concatenate([-sinH, sinH], 0)
    # fold per-dim rms weights into the rope tables: out row p uses
    # cos[p]*rw[p] and sin[p]*rw[swap(p)] (swap = +-64)
    qrwp = np.asarray(q_rms_w).astype(f32)[perm]
    krwp = np.asarray(k_rms_w).astype(f32)[perm]
    swap = np.concatenate([np.arange(64, 128), np.arange(0, 64)])
    cosq = _bf16(cosP * qrwp[:, None])
    sinq = _bf16(sinP * qrwp[swap][:, None])
    cosk = _bf16(cosP * krwp[:, None])
    sink = _bf16(sinP * krwp[swap][:, None])
    tembT = np.ascontiguousarray(
        np.asarray(temb).reshape(D).astype(f32).reshape(KT, 128).T)
    qwT_all = np.asarray(qw).T.astype(f32)
    kwT_all = np.asarray(kw).T.astype(f32)
    vwT_all = np.asarray(vw).T.astype(f32)
    mlpT_all = np.asarray(mlp_w).T.astype(f32)
    outT_all = np.asarray(out_w).T.astype(f32)
    normT_all = np.asarray(norm_w).T.astype(f32)
    outb_col = np.ascontiguousarray(
        np.asarray(out_b).astype(f32).reshape(DBLK, 128).T)

    in_maps = []
    for c in range(NC):
        heads = range(HPC * c, HPC * (c + 1))
        qk_cols = np.concatenate([h * DH + perm for h in heads])
        v_cols = np.concatenate([h * DH + np.arange(DH) for h in heads])
        ml_sl = slice(MPC * c, MPC * (c + 1))
        nm_sl = slice(NPC * c, NPC * (c + 1))
        out_rows = np.concatenate(
            [DQ * c + np.arange(DQ), D + MPC * c + np.arange(MPC)]
        )
        # xresT: [RSC, RWC, S] rows = feature RB*128*i + RWC*c + j
        xresT = np.empty((RSC, RWC, S), f32)
        for i in range(RSC):
            f0 = RB * 128 * i + RWC * c
            xresT[i] = xT[f0 : f0 + RWC, :]
        outTc = outT_all[out_rows, :]  # [FPC, D]
        outTd = np.ascontiguousarray(
            outTc.reshape(FT, 128, DBLK, 128).transpose(2, 1, 0, 3))
        mslab = mlpT_all[:, ml_sl].reshape(KT, 128, MT, 128)
        mslab = np.ascontiguousarray(
            mslab.transpose(2, 1, 0, 3).reshape(MT, 128, D))
        m = {
            "xTb": xTb,
            "xresT": xresT,
            "tembT": tembT,
            "outb_col": outb_col,
            "normT": _bf16(normT_all[:, nm_sl]),
            "normb_row": _bf16(
                np.asarray(norm_b).astype(f32)[nm_sl].reshape(1, NPC)),
            "qw8": _pack_pairs(qwT_all, qk_cols, WS),
            "kw8": _pack_pairs(kwT_all, qk_cols, WS),
            "vw8": _pack_pairs(vwT_all, v_cols, WS),
            "qb": np.ascontiguousarray(
                np.asarray(qb).astype(f32)[qk_cols].reshape(HPC, 128).T),
            "kb": np.ascontiguousarray(
                np.asarray(kb).astype(f32)[qk_cols].reshape(HPC, 128).T),
            "vb": np.ascontiguousarray(
                np.asarray(vb).astype(f32)[v_cols].reshape(HPC, 128).T),
            "mlpT": _bf16(mslab),
            "mlpb": np.ascontiguousarray(
                np.asarray(mlp_b).astype(f32)[ml_sl].reshape(MT, 128).T),
            "outTd": _bf16(outTd),
            "cosq": cosq,
            "sinq": sinq,
            "cosk": cosk,
            "sink": sink,
        }
        in_maps.append(m)
    return in_maps


def run(inputs, debug=False, trace=False):
    nc = _build(debug=debug)
    in_maps = _prep_inputs(**inputs)
    res = run_bass_kernel_spmd(nc, in_maps, list(range(NC)), trace=trace)
    out = np.empty((S, D), np.float32)
    for c in range(NC):
        ys = res.results[c]["y"]  # [RSC, RWC, S]
        for i in range(RSC):
            f0 = RB * 128 * i + RWC * c
            out[:, f0 : f0 + RWC] = ys[i].T
    return out.reshape(B, S, D), res


def kernel(**inputs):
    out, _ = run(inputs)
    return out
